# revision 58
# baseline (speedup 1.0000x reference)
"""Trainium2 Bass kernel for the recurrent STP network (nn_Network_20109036880204).

Strategy: tensor-parallel over the output-neuron dim across 8 NeuronCores.
  - Each core owns a 1024-neuron shard: W_c = Wab[c*1024:(c+1)*1024, :]^T,
    stored fp16 resident in SBUF as 64 K-tiles [128, 1024] (128 KiB/partition).
  - All [B, N] state tensors live in SBUF in "state layout": tile [128, 256]
    with  tile[p, j*32 + b] = state[b, n = c*1024 + j*128 + p].
  - Matmul uses PE col-group tiling: the stationary y K-tile is only 32 wide
    (batch), so 4 K-tiles run CONCURRENTLY in array col-groups s=0..3
    (tile_position=(0, 32s)), each accumulating into PSUM partitions
    [32s:32s+32) of a [128, 512] bank. The 4 partial strips are then reduced
    by one tiny "fold" matmul (lhsT = 4x-replicated identity * dt_tau_syn),
    transposed back to state layout by 4 PE transposes, and fed to the
    fused DVE update chain.
  - Per step: y = u'*x'*r (fp16) -> DRAM -> AllGather(8) -> y_full in SBUF,
    pipelined in two halves (A = j<4, B = j>=4) so the gathers hide under
    the matmuls.
"""

import sys

for _p in ("/opt/trn_rl_repo", "/root/.axon_site/_ro/trn_rl_repo"):
    if _p not in sys.path:
        sys.path.append(_p)

import ml_dtypes
import numpy as np

import concourse.bass as bass
import concourse.bacc as bacc
import concourse.mybir as mybir
import concourse.tile as tile
from concourse import bass_utils

# problem constants
NCORES = 8
B = 32
N = 8192
NS = N // NCORES          # 1024 neurons per core
P = 128
J = NS // P               # 8 local K-tiles per core
T = N // P                # 64 K-tiles total
F = J * B                 # 256 = free size of a state tile
CHUNK = 512               # matmul moving free dim (one PSUM bank)
NCH = NS // CHUNK         # 2 chunks
S = 4                     # concurrent PE col-group strips

DT = 0.01
USE = 0.03
TAU_FAC = 1.0
TAU_REC = 0.25
C1 = DT / TAU_FAC         # 0.01
C0 = DT * USE / TAU_FAC   # 3e-4
A1 = USE * DT             # 3e-4
C2 = DT / TAU_REC         # 0.04

F32 = mybir.dt.float32
F16 = mybir.dt.float16
F8 = mybir.dt.float8e4
MULT = mybir.AluOpType.mult
ADD = mybir.AluOpType.add
MAX = mybir.AluOpType.max

# y is carried in fp8 e4m3 scaled by YS (|y*YS| <= ~20 << 240): halves the
# AllGather payload and all y DMAs. The matmul is mixed-precision (fp8
# stationary y x fp16 moving W), so PSUM = YS*mm; the fold matrix carries
# ds/YS to undo it.
YS = 64.0

# A/B halves: half A = j-blocks 0..3 (gathered early), half B = j 4..7.
JA = 4
JB = J - JA
HA = JA * B               # 128 = state-free width of half A
HB = JB * B               # 128
HW = {"A": HA, "B": HB}
A_TILES = [t for t in range(T) if t % J < JA]
B_TILES = [t for t in range(T) if t % J >= JA]
EW_DVE = 64               # ew-chain cols on DVE; rest (HA-EW_DVE) on Pool


def build_program(n_steps: int, uni=(None, None, None, None), n_dummy=10):
    """Build the SPMD Bass program (identical on all 8 cores)."""
    es_v, ds_v, e_v, dt_v = uni  # uniform values of the const vectors, or None

    nc = bacc.Bacc(
        "TRN2",
        target_bir_lowering=False,
        debug=False,
        num_devices=NCORES,
    )

    w_dram = nc.dram_tensor("w", [T, P, NS], F16, kind="ExternalInput")
    sd = {
        nm: nc.dram_tensor(nm, [P, F], F32, kind="ExternalInput")
        for nm in ["r0", "recs0", "u0", "x0", "ff", "es", "ds", "e", "dt"]
    }
    fold_dram = nc.dram_tensor("fold", [P, B], F16, kind="ExternalInput")
    identh_dram = nc.dram_tensor("identh", [B, B], F16, kind="ExternalInput")
    r_out = nc.dram_tensor("r_out", [P, F], F32, kind="ExternalOutput")

    with tile.TileContext(nc) as tc:
        with (
            tc.tile_pool(name="wpool", bufs=1) as wpool,
            tc.tile_pool(name="cpool", bufs=1) as cpool,
            tc.tile_pool(name="spool", bufs=2) as spool,
            tc.tile_pool(name="wk", bufs=2) as wk,
            tc.tile_pool(name="yp", bufs=2) as yp,
            tc.tile_pool(name="pmm", bufs=2, space="PSUM") as pmm,
            tc.tile_pool(name="pT", bufs=2, space="PSUM") as pT,
            tc.tile_pool(name="dp", bufs=3, space="DRAM") as dp,
        ):
            # ---- resident weights: 16 DMAs so they spread across queues ----
            w_sb = wpool.tile([P, T * NS], F16, tag="w")
            TB = 4  # K-tiles per DMA
            for i in range(T // TB):
                dst = w_sb[:, i * TB * NS:(i + 1) * TB * NS].rearrange(
                    "p (t n) -> p t n", t=TB
                )
                src = w_dram[i * TB:(i + 1) * TB, :, :].rearrange("t p n -> p t n")
                nc.sync.dma_start(dst, src)

            # ---- constants / initial state ----
            ff_sb = cpool.tile([P, F], F32, tag="ff")
            es_sb = cpool.tile([P, F], F32, tag="es")
            ds_sb = cpool.tile([P, F], F32, tag="ds")
            e_sb = cpool.tile([P, F], F32, tag="e")
            dt_sb = cpool.tile([P, F], F32, tag="dt")
            fold_sb = cpool.tile([P, B], F16, tag="fold")
            identh = cpool.tile([B, B], F16, tag="identh")
            for t_, nm in [(ff_sb, "ff"), (es_sb, "es"), (ds_sb, "ds"),
                           (e_sb, "e"), (dt_sb, "dt")]:
                nc.sync.dma_start(t_[:], sd[nm][:])
            nc.sync.dma_start(fold_sb[:], fold_dram[:])
            nc.sync.dma_start(identh[:], identh_dram[:])

            r = spool.tile([P, F], F32, tag="r")
            recS = spool.tile([P, F], F32, tag="recS")
            u0_sb = wk.tile([P, F], F32, tag="u0", bufs=1)
            x0_sb = wk.tile([P, F], F32, tag="x0", bufs=1)
            for t_, nm in [(r, "r0"), (recS, "recs0"), (u0_sb, "u0"),
                           (x0_sb, "x0")]:
                nc.sync.dma_start(t_[:], sd[nm][:])

            V = nc.vector

            # Pool (nc.gpsimd) only supports tensor_tensor/tensor_scalar, so
            # the DVE/Pool-split ew chain needs TT/TS-only ops. To keep the
            # final y = YS*u*x*r a plain multiply, the x state is carried
            # PRE-SCALED as vY = YS*x when all rate constants are uniform.
            pool_split = (ds_v is not None and es_v is not None
                          and e_v is not None and dt_v is not None)
            XS = YS if pool_split else 1.0

            # ---- prologue: u1, x1 (or YS*x1), y0 from initial state ----
            s1 = wk.tile([P, F], F32, tag="t0", bufs=1)
            m = wk.tile([P, F], F32, tag="t1", bufs=1)
            s2 = wk.tile([P, F], F32, tag="t2", bufs=1)
            un = spool.tile([P, F], F32, tag="u")
            V.tensor_scalar(s1[:], u0_sb[:], 1.0 - C1, C0, MULT, ADD)
            V.tensor_mul(m[:], u0_sb[:], r[:])
            V.scalar_tensor_tensor(s2[:], r[:], A1, s1[:], MULT, ADD)
            V.scalar_tensor_tensor(un[:], m[:], -A1, s2[:], MULT, ADD)

            t2p = wk.tile([P, F], F32, tag="t3", bufs=1)
            t3p = wk.tile([P, F], F32, tag="t4", bufs=1)
            s4 = wk.tile([P, F], F32, tag="t5", bufs=1)
            xn = spool.tile([P, F], F32, tag="x")
            V.tensor_mul(t2p[:], x0_sb[:], r[:])
            V.tensor_mul(t3p[:], un[:], t2p[:])
            V.tensor_scalar(s4[:], x0_sb[:], XS * (1.0 - C2), XS * C2,
                            MULT, ADD)
            V.scalar_tensor_tensor(xn[:], t3p[:], -DT * XS, s4[:], MULT, ADD)

            w0 = wk.tile([P, F], F32, tag="t6", bufs=1)
            yh = {}
            V.tensor_mul(w0[:], un[:], xn[:])
            for hf, sl in (("A", slice(0, HA)), ("B", slice(HA, F))):
                yh[hf] = yp.tile([P, HW[hf]], F8, tag=f"y{hf}",
                                 name=f"y{hf}_pro")
                V.scalar_tensor_tensor(yh[hf][:], w0[:, sl], YS / XS,
                                       r[:, sl], MULT, MULT)

            ag_counter = [0]

            def ag_send(hf, ytile):
                """store y-half to DRAM (from GpSimd, which also triggers)
                and launch the AllGather. The A and B gathers go on
                different CC streams so they can run concurrently instead
                of serializing on stream 0 (~8us apart)."""
                k = ag_counter[0] = ag_counter[0] + 1
                w_ = HW[hf]
                ydr = dp.tile([P, w_], F8, tag=f"ydr{hf}", name=f"ydr{hf}_{k}")
                nc.gpsimd.dma_start(ydr[:], ytile[:])
                yall = dp.tile([NCORES, P, w_], F8, tag=f"yall{hf}",
                               name=f"yall{hf}_{k}")
                cc = nc.gpsimd.collective_compute(
                    "AllGather",
                    mybir.AluOpType.bypass,
                    replica_groups=[list(range(NCORES))],
                    ins=[ydr.opt()],
                    outs=[yall.opt()],
                )
                del cc  # stream_id=1 for B was rejected by the runtime
                return yall, k

            def ag_recv(hf, yall, k):
                """DMA the gathered tiles back to SBUF, all from Sync.
                Keeping the stores + triggers alone on GpSimd means the B
                trigger is never queued behind an AG-A-done wait (the tile
                scheduler orders same-engine AG-done waits ahead of later
                emissions, so mixing returns onto GpSimd stalls trigB)."""
                w_ = HW[hf]
                yfull = yp.tile([P, NCORES * w_], F8, tag=f"yfull{hf}",
                                name=f"yfull{hf}_{k}")
                for lo, hi in ((0, 1), (1, 3), (3, 5), (5, 7), (7, 8)):
                    nc.sync.dma_start(
                        yfull[:, lo * w_:hi * w_].rearrange(
                            "p (c f) -> p c f", c=hi - lo),
                        yall[lo:hi, :, :].rearrange("c p f -> p c f"),
                    )
                return yfull

            sendA = ag_send("A", yh["A"])
            sendB = ag_send("B", yh["B"])
            yfullA = ag_recv("A", *sendA)
            yfullB = ag_recv("B", *sendB)

            # fast rec path: h = mm + (recS + ff) hoists one op off the
            # y-producing chain; recS' = es*h - es*ff with -es*ff constant
            fast_rec = ds_v is not None and es_v is not None
            if fast_rec:
                negesff = cpool.tile([P, F], F32, tag="negesff")
                V.tensor_scalar(negesff[:], ff_sb[:], -es_v, None, MULT)

            pdum = pmm.tile([B, CHUNK], F32, tag="dummy", bufs=1,
                            name="pdum") if n_dummy else None

            def lhst_ap(yfA, yfB, t):
                c, j = divmod(t, J)
                if j < JA:
                    return yfA[:, c * HA + j * B:c * HA + (j + 1) * B]
                jb = j - JA
                return yfB[:, c * HB + jb * B:c * HB + (jb + 1) * B]

            # ---- main loop ----
            for it in range(n_steps):
                last = it == n_steps - 1

                # precompute (overlaps AG + matmul on DVE)
                A_t = wk.tile([P, F], F32, tag="A", bufs=1)
                B_t = wk.tile([P, F], F32, tag="B", bufs=1)
                C_t = wk.tile([P, F], F32, tag="C", bufs=1)
                D_t = wk.tile([P, F], F32, tag="D", bufs=1)
                rE = wk.tile([P, F], F32, tag="rE", bufs=1)
                rff = wk.tile([P, F], F32, tag="rff", bufs=2,
                              name=f"rff_{it}")
                if fast_rec:
                    V.tensor_add(rff[:], recS[:], ff_sb[:])
                if not last:
                    V.tensor_scalar(A_t[:], un[:], 1.0 - C1, C0, MULT, ADD)
                    V.tensor_scalar(B_t[:], un[:], -A1, A1, MULT, ADD)
                    if pool_split:
                        # xn holds vY = YS*x: C_t := YS*C = vY*(1-C2)+YS*C2,
                        # D_t := -YS*D = -DT*vY
                        V.tensor_scalar(C_t[:], xn[:], 1.0 - C2, YS * C2,
                                        MULT, ADD)
                        V.tensor_scalar(D_t[:], xn[:], -DT, None, MULT)
                    else:
                        V.tensor_scalar(C_t[:], xn[:], 1.0 - C2, C2,
                                        MULT, ADD)
                        V.tensor_scalar(D_t[:], xn[:], DT, None, MULT)
                if e_v is None:
                    V.tensor_mul(rE[:], r[:], e_sb[:])
                elif pool_split:
                    V.tensor_scalar(rE[:], r[:], e_v, None, MULT)

                # dummy matmuls: PE/HAM-warming filler during the AllGather
                # wait at the head of each step (read-only on w_sb)
                if n_dummy and it > 0:
                    for _ in range(n_dummy):
                        nc.tensor.matmul(
                            pdum[:], lhsT=w_sb[:, :B], rhs=w_sb[:, :CHUNK],
                            start=True, stop=True,
                        )

                # matmul: 2 output chunk banks x 64 K-tiles; K-tile t runs in
                # PE col-group t%4, accumulating into PSUM partitions
                # [32s:32s+32). Chunks interleave per K-tile so both PSUM
                # banks stream concurrently. A-sourced K-tiles first so AG_B
                # can land late.
                pm = [pmm.tile([P, CHUNK], F32, tag=f"mm{ch}",
                               name=f"pm{ch}_{it}", bufs=2)
                      for ch in range(NCH)]
                nmm = [[0] * S for _ in range(NCH)]

                def emit(tiles, chs):
                    for t in tiles:
                        s = t % S
                        for ch in chs:
                            nc.tensor.matmul(
                                pm[ch][B * s:B * (s + 1), :],
                                lhsT=lhst_ap(yfullA, yfullB, t),
                                rhs=w_sb[:, t * NS + ch * CHUNK:
                                         t * NS + (ch + 1) * CHUNK],
                                start=(nmm[ch][s] == 0),
                                stop=(nmm[ch][s] == T // S - 1),
                                tile_position=(0, B * s),
                            )
                            nmm[ch][s] += 1

                QW = CHUNK // 2

                def reduce_chunk(ch, hf):
                    """PSUM chunk [128, 512] -> fp16 stage [128, 512].
                    Halves staged on ACT and DVE in parallel so the fold
                    matmuls (split to match) start ~2x earlier."""
                    stage4 = wk.tile([P, CHUNK], F16, tag=f"s4{hf}", bufs=2,
                                     name=f"s4{hf}_{it}")
                    nc.scalar.copy(stage4[:, :QW], pm[ch][:, :QW])
                    V.tensor_scalar(stage4[:, QW:], pm[ch][:, QW:], 1.0,
                                    None, MULT)
                    return stage4

                def fold_chunk(ch, hf, stage4):
                    pfold = pT.tile([B, CHUNK], F32, tag="pfold", bufs=1,
                                    name=f"pfold{hf}_{it}")
                    stage2 = wk.tile([B, CHUNK], F16, tag=f"s2{hf}", bufs=2,
                                     name=f"s2{hf}_{it}")
                    nc.tensor.matmul(pfold[:, :QW], lhsT=fold_sb[:],
                                     rhs=stage4[:, :QW], start=True, stop=True)
                    nc.scalar.copy(stage2[:, :QW], pfold[:, :QW])
                    nc.tensor.matmul(pfold[:, QW:], lhsT=fold_sb[:],
                                     rhs=stage4[:, QW:], start=True, stop=True)
                    V.tensor_scalar(stage2[:, QW:], pfold[:, QW:], 1.0,
                                    None, MULT)
                    return stage2

                def transpose_chunk(hf, stage2):
                    mmT_ = pT.tile([P, HW[hf]], F16, tag=f"mmT{hf}", bufs=1,
                                   name=f"mmT{hf}_{it}")
                    for j in range(JA):
                        nc.tensor.transpose(
                            mmT_[:, j * B:(j + 1) * B],
                            stage2[:, j * P:(j + 1) * P],
                            identh[:],
                        )
                    return mmT_

                emit(A_TILES, (0, 1))
                emit(B_TILES, (0,))
                s4A = reduce_chunk(0, "A")
                emit(B_TILES[:8], (1,))
                s2A = fold_chunk(0, "A", s4A)
                emit(B_TILES[8:16], (1,))
                mmTA = transpose_chunk("A", s2A)
                emit(B_TILES[16:], (1,))

                # names for per-half state pieces of this iteration
                rec_new = spool.tile([P, F], F32, tag="recfull")
                r_new = spool.tile([P, F], F32, tag="r")
                recS_new = spool.tile([P, F], F32, tag="recS")
                q = spool.tile([P, F], F32, tag="u")
                v = spool.tile([P, F], F32, tag="x")
                newy = {"A": yp.tile([P, HA], F8, tag="yA", name=f"yA_{it}"),
                        "B": yp.tile([P, HB], F8, tag="yB", name=f"yB_{it}")}

                def ew_half(hf, mmT_half):
                    """Element-wise update chain, column-split across DVE
                    (cols 0:EW_DVE) and Pool (cols EW_DVE:) so the two
                    engines run the serial chain concurrently."""
                    sl = slice(0, HA) if hf == "A" else slice(HA, F)
                    HF = HW[hf]
                    h_ = wk.tile([P, HF], F32, tag=f"w1{hf}", bufs=1)
                    dr_ = wk.tile([P, HF], F32, tag=f"w2{hf}", bufs=1)
                    m1_ = wk.tile([P, HF], F32, tag=f"w3{hf}", bufs=1)
                    tt_ = wk.tile([P, HF], F32, tag=f"w4{hf}", bufs=1)
                    s2_ = wk.tile([P, HF], F32, tag=f"w5{hf}", bufs=1)
                    tmp = wk.tile([P, HF], F32, tag=f"w0{hf}", bufs=1)
                    ynew = None if last else newy[hf]

                    # h reads mmT from PSUM, which Pool cannot access --
                    # compute it (and the general-path rec chain) on DVE
                    # full-width, then split the rest across DVE/Pool.
                    if fast_rec:
                        V.tensor_add(h_[:], mmT_half[:], rff[:, sl])
                    elif ds_v is not None:
                        V.tensor_add(rec_new[:, sl], mmT_half[:], recS[:, sl])
                        V.tensor_add(h_[:], rec_new[:, sl], ff_sb[:, sl])
                    else:
                        V.tensor_mul(tmp[:], mmT_half[:], ds_sb[:, sl])
                        V.tensor_add(rec_new[:, sl], tmp[:], recS[:, sl])
                        V.tensor_add(h_[:], rec_new[:, sl], ff_sb[:, sl])

                    def chain(E, i0, i1):
                        """TT/TS-only chain (Pool-compatible). C_t/D_t hold
                        YS*C and -YS*D, and v holds vY = YS*x, so y is the
                        plain product tt*vY."""
                        il = slice(i0, i1)           # intermediate cols
                        gl = slice(sl.start + i0, sl.start + i1)  # state cols
                        E.tensor_scalar(dr_[:, il], h_[:, il], 0.0, dt_v,
                                        MAX, MULT)
                        E.tensor_add(r_new[:, gl], dr_[:, il], rE[:, gl])
                        if last:
                            return
                        E.tensor_mul(m1_[:, il], B_t[:, gl], r_new[:, gl])
                        E.tensor_add(q[:, gl], m1_[:, il], A_t[:, gl])
                        E.tensor_mul(tt_[:, il], r_new[:, gl], q[:, gl])
                        E.tensor_mul(s2_[:, il], D_t[:, gl], tt_[:, il])
                        E.tensor_add(v[:, gl], s2_[:, il], C_t[:, gl])
                        E.tensor_mul(ynew[:, il], tt_[:, il], v[:, gl])

                    def chain_dve(i0, i1):
                        """general-path chain on DVE only (STT forms)."""
                        il = slice(i0, i1)
                        gl = slice(sl.start + i0, sl.start + i1)
                        if dt_v is not None:
                            V.tensor_scalar(dr_[:, il], h_[:, il], 0.0, dt_v,
                                            MAX, MULT)
                        else:
                            V.scalar_tensor_tensor(dr_[:, il], h_[:, il], 0.0,
                                                   dt_sb[:, gl], MAX, MULT)
                        if e_v is not None:
                            V.scalar_tensor_tensor(r_new[:, gl], r[:, gl],
                                                   e_v, dr_[:, il], MULT, ADD)
                        else:
                            V.tensor_add(r_new[:, gl], dr_[:, il], rE[:, gl])
                        if last:
                            return
                        V.tensor_mul(m1_[:, il], B_t[:, gl], r_new[:, gl])
                        V.tensor_add(q[:, gl], m1_[:, il], A_t[:, gl])
                        V.tensor_mul(tt_[:, il], r_new[:, gl], q[:, gl])
                        V.tensor_mul(s2_[:, il], D_t[:, gl], tt_[:, il])
                        V.scalar_tensor_tensor(v[:, gl], s2_[:, il], -1.0,
                                               C_t[:, gl], MULT, ADD)
                        V.scalar_tensor_tensor(ynew[:, il], tt_[:, il], YS,
                                               v[:, gl], MULT, MULT)

                    if pool_split:
                        chain(V, 0, EW_DVE)
                        chain(nc.gpsimd, EW_DVE, HF)
                    else:
                        chain_dve(0, HF)
                    # off the y critical path: recS' update, full-width DVE
                    if not last:
                        if fast_rec:
                            # recS' = es*(h - ff) = es*h + (-es*ff)
                            V.scalar_tensor_tensor(recS_new[:, sl], h_[:],
                                                   es_v, negesff[:, sl],
                                                   MULT, ADD)
                        elif es_v is not None:
                            V.tensor_scalar(recS_new[:, sl], rec_new[:, sl],
                                            es_v, None, MULT)
                        else:
                            V.tensor_mul(recS_new[:, sl], rec_new[:, sl],
                                         es_sb[:, sl])
                    return ynew

                yA_next = ew_half("A", mmTA)
                if not last:
                    sA = ag_send("A", yA_next)

                s4B = reduce_chunk(1, "B")
                s2B = fold_chunk(1, "B", s4B)
                mmTB = transpose_chunk("B", s2B)
                yB_next = ew_half("B", mmTB)
                if not last:
                    sB = ag_send("B", yB_next)
                    yfullA = ag_recv("A", *sA)
                    yfullB = ag_recv("B", *sB)
                    un, xn, recS = q, v, recS_new
                r = r_new

            # ---- epilogue ----
            for qi in range(4):
                nc.sync.dma_start(
                    r_out[32 * qi:32 * (qi + 1), :],
                    r[32 * qi:32 * (qi + 1), :],
                )

    nc.compile()
    return nc


# ---------------------------------------------------------------------------
# host-side data marshalling
# ---------------------------------------------------------------------------

def _shard_state(v, c):
    """[B, N] float array -> core c state tile [128, 256] (f32)."""
    vs = np.asarray(v, np.float32)[:, c * NS:(c + 1) * NS]      # [32, 1024]
    return np.ascontiguousarray(
        vs.reshape(B, J, P).transpose(2, 1, 0).reshape(P, F)
    )


def _shard_vec(v, c):
    """[N] float vector -> replicated core c tile [128, 256] (f32)."""
    vs = np.asarray(v, np.float32)[c * NS:(c + 1) * NS].reshape(J, P)  # [j, p]
    t = vs.T[:, :, None]                                        # [p, j, 1]
    return np.ascontiguousarray(np.broadcast_to(t, (P, J, B)).reshape(P, F))


def _shard_w(Wab, c):
    """Wab [N, N] -> core c weight tiles [64, 128, 1024] fp16.

    w[t, p, n] = Wab[c*1024 + n, t*128 + p]
    """
    wt = np.asarray(Wab, np.float32)[c * NS:(c + 1) * NS, :].T  # [8192, 1024]
    return np.ascontiguousarray(wt.astype(np.float16).reshape(T, P, NS))


def _fold_mat(ds_v):
    """[128, 32] fp16: fold[32s+m, m] = ds/YS (strip-sum + ds scale +
    removal of the fp8 y scale YS)."""
    f = np.zeros((P, B), np.float16)
    scale = np.float16((1.0 if ds_v is None else ds_v) / YS)
    for s in range(S):
        f[s * B:(s + 1) * B, :][np.diag_indices(B)] = scale
    return f


def _unshard_out(tiles):
    """list of 8 [128, 256] tiles -> [32, 8192] f32."""
    out = np.empty((B, N), np.float32)
    for c, tl in enumerate(tiles):
        out[:, c * NS:(c + 1) * NS] = (
            np.asarray(tl, np.float32).reshape(P, J, B).transpose(2, 1, 0)
            .reshape(B, NS)
        )
    return out


def make_in_maps(rates, rec_input, ff_input, Wab, u_stp, x_stp,
                 exp_dt_tau, dt_tau, exp_dt_tau_syn, dt_tau_syn):
    recs_full = (np.asarray(exp_dt_tau_syn, np.float32)[None, :]
                 * np.asarray(rec_input, np.float32))
    ds_v = _uniform_val(dt_tau_syn)
    fold = _fold_mat(ds_v)
    identh = np.eye(B, dtype=np.float16)
    in_maps = []
    for c in range(NCORES):
        in_maps.append({
            "w": _shard_w(Wab, c),
            "r0": _shard_state(rates, c),
            "recs0": _shard_state(recs_full, c),
            "u0": _shard_state(u_stp, c),
            "x0": _shard_state(x_stp, c),
            "ff": _shard_state(ff_input, c),
            "es": _shard_vec(exp_dt_tau_syn, c),
            "ds": _shard_vec(dt_tau_syn, c),
            "e": _shard_vec(exp_dt_tau, c),
            "dt": _shard_vec(dt_tau, c),
            "fold": fold,
            "identh": identh,
        })
    return in_maps


_PROGRAM_CACHE = {}


def _uniform_val(v):
    v = np.asarray(v, np.float32)
    return float(v.flat[0]) if np.all(v == v.flat[0]) else None


def _get_program(n_steps, uni):
    key = (n_steps, uni)
    if key not in _PROGRAM_CACHE:
        _PROGRAM_CACHE[key] = build_program(n_steps, uni=uni)
    return _PROGRAM_CACHE[key]


def run(trace=False, tmpdir=None, **inputs):
    n_steps = int(inputs.pop("n_steps"))
    uni = (_uniform_val(inputs["exp_dt_tau_syn"]),
           _uniform_val(inputs["dt_tau_syn"]),
           _uniform_val(inputs["exp_dt_tau"]),
           _uniform_val(inputs["dt_tau"]))
    nc = _get_program(n_steps, uni)
    in_maps = make_in_maps(**inputs)
    res = bass_utils.run_bass_kernel_spmd(
        nc, in_maps, core_ids=list(range(NCORES)), trace=trace, tmpdir=tmpdir
    )
    out = _unshard_out([m["r_out"] for m in res.results])
    return out, res


def kernel(**inputs):
    out, _ = run(**inputs)
    return out



# revision 60
# speedup vs baseline: 1.0008x; 1.0008x over previous
"""Trainium2 Bass kernel for the recurrent STP network (nn_Network_20109036880204).

Strategy: tensor-parallel over the output-neuron dim across 8 NeuronCores.
  - Each core owns a 1024-neuron shard: W_c = Wab[c*1024:(c+1)*1024, :]^T,
    stored fp16 resident in SBUF as 64 K-tiles [128, 1024] (128 KiB/partition).
  - All [B, N] state tensors live in SBUF in "state layout": tile [128, 256]
    with  tile[p, j*32 + b] = state[b, n = c*1024 + j*128 + p].
  - Matmul uses PE col-group tiling: the stationary y K-tile is only 32 wide
    (batch), so 4 K-tiles run CONCURRENTLY in array col-groups s=0..3
    (tile_position=(0, 32s)), each accumulating into PSUM partitions
    [32s:32s+32) of a [128, 512] bank. The 4 partial strips are then reduced
    by one tiny "fold" matmul (lhsT = 4x-replicated identity * dt_tau_syn),
    transposed back to state layout by 4 PE transposes, and fed to the
    fused DVE update chain.
  - Per step: y = u'*x'*r (fp16) -> DRAM -> AllGather(8) -> y_full in SBUF,
    pipelined in two halves (A = j<4, B = j>=4) so the gathers hide under
    the matmuls.
"""

import sys

for _p in ("/opt/trn_rl_repo", "/root/.axon_site/_ro/trn_rl_repo"):
    if _p not in sys.path:
        sys.path.append(_p)

import ml_dtypes
import numpy as np

import concourse.bass as bass
import concourse.bacc as bacc
import concourse.mybir as mybir
import concourse.tile as tile
from concourse import bass_utils

# problem constants
NCORES = 8
B = 32
N = 8192
NS = N // NCORES          # 1024 neurons per core
P = 128
J = NS // P               # 8 local K-tiles per core
T = N // P                # 64 K-tiles total
F = J * B                 # 256 = free size of a state tile
CHUNK = 512               # matmul moving free dim (one PSUM bank)
NCH = NS // CHUNK         # 2 chunks
S = 4                     # concurrent PE col-group strips

DT = 0.01
USE = 0.03
TAU_FAC = 1.0
TAU_REC = 0.25
C1 = DT / TAU_FAC         # 0.01
C0 = DT * USE / TAU_FAC   # 3e-4
A1 = USE * DT             # 3e-4
C2 = DT / TAU_REC         # 0.04

F32 = mybir.dt.float32
F16 = mybir.dt.float16
F8 = mybir.dt.float8e4
MULT = mybir.AluOpType.mult
ADD = mybir.AluOpType.add
MAX = mybir.AluOpType.max

# y is carried in fp8 e4m3 scaled by YS (|y*YS| <= ~20 << 240): halves the
# AllGather payload and all y DMAs. The matmul is mixed-precision (fp8
# stationary y x fp16 moving W), so PSUM = YS*mm; the fold matrix carries
# ds/YS to undo it.
YS = 64.0

# A/B halves: half A = j-blocks 0..3 (gathered early), half B = j 4..7.
JA = 4
JB = J - JA
HA = JA * B               # 128 = state-free width of half A
HB = JB * B               # 128
HW = {"A": HA, "B": HB}
A_TILES = [t for t in range(T) if t % J < JA]
B_TILES = [t for t in range(T) if t % J >= JA]
EW_DVE = 128              # ew-chain cols on DVE; rest on Pool (Pool ops
                          # measured 320-1100ns vs DVE ~225 -> keep all DVE)


def build_program(n_steps: int, uni=(None, None, None, None), n_dummy=10):
    """Build the SPMD Bass program (identical on all 8 cores)."""
    es_v, ds_v, e_v, dt_v = uni  # uniform values of the const vectors, or None

    nc = bacc.Bacc(
        "TRN2",
        target_bir_lowering=False,
        debug=False,
        num_devices=NCORES,
    )

    w_dram = nc.dram_tensor("w", [T, P, NS], F16, kind="ExternalInput")
    sd = {
        nm: nc.dram_tensor(nm, [P, F], F32, kind="ExternalInput")
        for nm in ["r0", "recs0", "u0", "x0", "ff", "es", "ds", "e", "dt"]
    }
    fold_dram = nc.dram_tensor("fold", [P, B], F16, kind="ExternalInput")
    identh_dram = nc.dram_tensor("identh", [B, B], F16, kind="ExternalInput")
    r_out = nc.dram_tensor("r_out", [P, F], F32, kind="ExternalOutput")

    with tile.TileContext(nc) as tc:
        with (
            tc.tile_pool(name="wpool", bufs=1) as wpool,
            tc.tile_pool(name="cpool", bufs=1) as cpool,
            tc.tile_pool(name="spool", bufs=2) as spool,
            tc.tile_pool(name="wk", bufs=2) as wk,
            tc.tile_pool(name="yp", bufs=2) as yp,
            tc.tile_pool(name="pmm", bufs=2, space="PSUM") as pmm,
            tc.tile_pool(name="pT", bufs=2, space="PSUM") as pT,
            tc.tile_pool(name="dp", bufs=3, space="DRAM") as dp,
        ):
            # ---- resident weights: 16 DMAs so they spread across queues ----
            w_sb = wpool.tile([P, T * NS], F16, tag="w")
            TB = 4  # K-tiles per DMA
            for i in range(T // TB):
                dst = w_sb[:, i * TB * NS:(i + 1) * TB * NS].rearrange(
                    "p (t n) -> p t n", t=TB
                )
                src = w_dram[i * TB:(i + 1) * TB, :, :].rearrange("t p n -> p t n")
                nc.sync.dma_start(dst, src)

            # ---- constants / initial state ----
            ff_sb = cpool.tile([P, F], F32, tag="ff")
            es_sb = cpool.tile([P, F], F32, tag="es")
            ds_sb = cpool.tile([P, F], F32, tag="ds")
            e_sb = cpool.tile([P, F], F32, tag="e")
            dt_sb = cpool.tile([P, F], F32, tag="dt")
            fold_sb = cpool.tile([P, B], F16, tag="fold")
            identh = cpool.tile([B, B], F16, tag="identh")
            for t_, nm in [(ff_sb, "ff"), (es_sb, "es"), (ds_sb, "ds"),
                           (e_sb, "e"), (dt_sb, "dt")]:
                nc.sync.dma_start(t_[:], sd[nm][:])
            nc.sync.dma_start(fold_sb[:], fold_dram[:])
            nc.sync.dma_start(identh[:], identh_dram[:])

            r = spool.tile([P, F], F32, tag="r")
            recS = spool.tile([P, F], F32, tag="recS")
            u0_sb = wk.tile([P, F], F32, tag="u0", bufs=1)
            x0_sb = wk.tile([P, F], F32, tag="x0", bufs=1)
            for t_, nm in [(r, "r0"), (recS, "recs0"), (u0_sb, "u0"),
                           (x0_sb, "x0")]:
                nc.sync.dma_start(t_[:], sd[nm][:])

            V = nc.vector

            # Pool (nc.gpsimd) only supports tensor_tensor/tensor_scalar, so
            # the DVE/Pool-split ew chain needs TT/TS-only ops. To keep the
            # final y = YS*u*x*r a plain multiply, the x state is carried
            # PRE-SCALED as vY = YS*x when all rate constants are uniform.
            pool_split = (ds_v is not None and es_v is not None
                          and e_v is not None and dt_v is not None)
            XS = YS if pool_split else 1.0

            # ---- prologue: u1, x1 (or YS*x1), y0 from initial state ----
            s1 = wk.tile([P, F], F32, tag="t0", bufs=1)
            m = wk.tile([P, F], F32, tag="t1", bufs=1)
            s2 = wk.tile([P, F], F32, tag="t2", bufs=1)
            un = spool.tile([P, F], F32, tag="u")
            V.tensor_scalar(s1[:], u0_sb[:], 1.0 - C1, C0, MULT, ADD)
            V.tensor_mul(m[:], u0_sb[:], r[:])
            V.scalar_tensor_tensor(s2[:], r[:], A1, s1[:], MULT, ADD)
            V.scalar_tensor_tensor(un[:], m[:], -A1, s2[:], MULT, ADD)

            t2p = wk.tile([P, F], F32, tag="t3", bufs=1)
            t3p = wk.tile([P, F], F32, tag="t4", bufs=1)
            s4 = wk.tile([P, F], F32, tag="t5", bufs=1)
            xn = spool.tile([P, F], F32, tag="x")
            V.tensor_mul(t2p[:], x0_sb[:], r[:])
            V.tensor_mul(t3p[:], un[:], t2p[:])
            V.tensor_scalar(s4[:], x0_sb[:], XS * (1.0 - C2), XS * C2,
                            MULT, ADD)
            V.scalar_tensor_tensor(xn[:], t3p[:], -DT * XS, s4[:], MULT, ADD)

            w0 = wk.tile([P, F], F32, tag="t6", bufs=1)
            yh = {}
            V.tensor_mul(w0[:], un[:], xn[:])
            for hf, sl in (("A", slice(0, HA)), ("B", slice(HA, F))):
                yh[hf] = yp.tile([P, HW[hf]], F8, tag=f"y{hf}",
                                 name=f"y{hf}_pro")
                V.scalar_tensor_tensor(yh[hf][:], w0[:, sl], YS / XS,
                                       r[:, sl], MULT, MULT)

            ag_counter = [0]

            def ag_send(hf, ytile):
                """store y-half to DRAM (from GpSimd, which also triggers)
                and launch the AllGather. The A and B gathers go on
                different CC streams so they can run concurrently instead
                of serializing on stream 0 (~8us apart)."""
                k = ag_counter[0] = ag_counter[0] + 1
                w_ = HW[hf]
                ydr = dp.tile([P, w_], F8, tag=f"ydr{hf}", name=f"ydr{hf}_{k}")
                nc.gpsimd.dma_start(ydr[:], ytile[:])
                yall = dp.tile([NCORES, P, w_], F8, tag=f"yall{hf}",
                               name=f"yall{hf}_{k}")
                cc = nc.gpsimd.collective_compute(
                    "AllGather",
                    mybir.AluOpType.bypass,
                    replica_groups=[list(range(NCORES))],
                    ins=[ydr.opt()],
                    outs=[yall.opt()],
                )
                del cc  # stream_id=1 for B was rejected by the runtime
                return yall, k

            def ag_recv(hf, yall, k):
                """DMA the gathered tiles back to SBUF, all from Sync.
                Keeping the stores + triggers alone on GpSimd means the B
                trigger is never queued behind an AG-A-done wait (the tile
                scheduler orders same-engine AG-done waits ahead of later
                emissions, so mixing returns onto GpSimd stalls trigB)."""
                w_ = HW[hf]
                yfull = yp.tile([P, NCORES * w_], F8, tag=f"yfull{hf}",
                                name=f"yfull{hf}_{k}")
                for lo, hi in ((0, 1), (1, 3), (3, 5), (5, 7), (7, 8)):
                    nc.sync.dma_start(
                        yfull[:, lo * w_:hi * w_].rearrange(
                            "p (c f) -> p c f", c=hi - lo),
                        yall[lo:hi, :, :].rearrange("c p f -> p c f"),
                    )
                return yfull

            sendA = ag_send("A", yh["A"])
            sendB = ag_send("B", yh["B"])
            yfullA = ag_recv("A", *sendA)
            yfullB = ag_recv("B", *sendB)

            # fast rec path: h = mm + (recS + ff) hoists one op off the
            # y-producing chain; recS' = es*h - es*ff with -es*ff constant
            fast_rec = ds_v is not None and es_v is not None
            if fast_rec:
                negesff = cpool.tile([P, F], F32, tag="negesff")
                V.tensor_scalar(negesff[:], ff_sb[:], -es_v, None, MULT)

            pdum = pmm.tile([B, CHUNK], F32, tag="dummy", bufs=1,
                            name="pdum") if n_dummy else None

            def lhst_ap(yfA, yfB, t):
                c, j = divmod(t, J)
                if j < JA:
                    return yfA[:, c * HA + j * B:c * HA + (j + 1) * B]
                jb = j - JA
                return yfB[:, c * HB + jb * B:c * HB + (jb + 1) * B]

            # ---- main loop ----
            for it in range(n_steps):
                last = it == n_steps - 1

                # precompute (overlaps AG + matmul on DVE)
                A_t = wk.tile([P, F], F32, tag="A", bufs=1)
                B_t = wk.tile([P, F], F32, tag="B", bufs=1)
                C_t = wk.tile([P, F], F32, tag="C", bufs=1)
                D_t = wk.tile([P, F], F32, tag="D", bufs=1)
                rE = wk.tile([P, F], F32, tag="rE", bufs=1)
                rff = wk.tile([P, F], F32, tag="rff", bufs=2,
                              name=f"rff_{it}")
                if fast_rec:
                    V.tensor_add(rff[:], recS[:], ff_sb[:])
                if not last:
                    V.tensor_scalar(A_t[:], un[:], 1.0 - C1, C0, MULT, ADD)
                    V.tensor_scalar(B_t[:], un[:], -A1, A1, MULT, ADD)
                    if pool_split:
                        # xn holds vY = YS*x: C_t := YS*C = vY*(1-C2)+YS*C2,
                        # D_t := -YS*D = -DT*vY
                        V.tensor_scalar(C_t[:], xn[:], 1.0 - C2, YS * C2,
                                        MULT, ADD)
                        V.tensor_scalar(D_t[:], xn[:], -DT, None, MULT)
                    else:
                        V.tensor_scalar(C_t[:], xn[:], 1.0 - C2, C2,
                                        MULT, ADD)
                        V.tensor_scalar(D_t[:], xn[:], DT, None, MULT)
                if e_v is None:
                    V.tensor_mul(rE[:], r[:], e_sb[:])
                elif pool_split:
                    V.tensor_scalar(rE[:], r[:], e_v, None, MULT)

                # dummy matmuls: PE/HAM-warming filler during the AllGather
                # wait at the head of each step (read-only on w_sb)
                if n_dummy and it > 0:
                    for _ in range(n_dummy):
                        nc.tensor.matmul(
                            pdum[:], lhsT=w_sb[:, :B], rhs=w_sb[:, :CHUNK],
                            start=True, stop=True,
                        )

                # matmul: 2 output chunk banks x 64 K-tiles; K-tile t runs in
                # PE col-group t%4, accumulating into PSUM partitions
                # [32s:32s+32). Chunks interleave per K-tile so both PSUM
                # banks stream concurrently. A-sourced K-tiles first so AG_B
                # can land late.
                pm = [pmm.tile([P, CHUNK], F32, tag=f"mm{ch}",
                               name=f"pm{ch}_{it}", bufs=2)
                      for ch in range(NCH)]
                nmm = [[0] * S for _ in range(NCH)]

                def emit(tiles, chs):
                    for t in tiles:
                        s = t % S
                        for ch in chs:
                            nc.tensor.matmul(
                                pm[ch][B * s:B * (s + 1), :],
                                lhsT=lhst_ap(yfullA, yfullB, t),
                                rhs=w_sb[:, t * NS + ch * CHUNK:
                                         t * NS + (ch + 1) * CHUNK],
                                start=(nmm[ch][s] == 0),
                                stop=(nmm[ch][s] == T // S - 1),
                                tile_position=(0, B * s),
                            )
                            nmm[ch][s] += 1

                QW = CHUNK // 2

                def reduce_chunk(ch, hf):
                    """PSUM chunk [128, 512] -> fp16 stage [128, 512].
                    Halves staged on ACT and DVE in parallel so the fold
                    matmuls (split to match) start ~2x earlier."""
                    stage4 = wk.tile([P, CHUNK], F16, tag=f"s4{hf}", bufs=2,
                                     name=f"s4{hf}_{it}")
                    nc.scalar.copy(stage4[:, :QW], pm[ch][:, :QW])
                    V.tensor_scalar(stage4[:, QW:], pm[ch][:, QW:], 1.0,
                                    None, MULT)
                    return stage4

                def fold_chunk(ch, hf, stage4):
                    pfold = pT.tile([B, CHUNK], F32, tag="pfold", bufs=1,
                                    name=f"pfold{hf}_{it}")
                    stage2 = wk.tile([B, CHUNK], F16, tag=f"s2{hf}", bufs=2,
                                     name=f"s2{hf}_{it}")
                    nc.tensor.matmul(pfold[:, :QW], lhsT=fold_sb[:],
                                     rhs=stage4[:, :QW], start=True, stop=True)
                    nc.scalar.copy(stage2[:, :QW], pfold[:, :QW])
                    nc.tensor.matmul(pfold[:, QW:], lhsT=fold_sb[:],
                                     rhs=stage4[:, QW:], start=True, stop=True)
                    V.tensor_scalar(stage2[:, QW:], pfold[:, QW:], 1.0,
                                    None, MULT)
                    return stage2

                def transpose_chunk(hf, stage2):
                    mmT_ = pT.tile([P, HW[hf]], F16, tag=f"mmT{hf}", bufs=1,
                                   name=f"mmT{hf}_{it}")
                    for j in range(JA):
                        nc.tensor.transpose(
                            mmT_[:, j * B:(j + 1) * B],
                            stage2[:, j * P:(j + 1) * P],
                            identh[:],
                        )
                    return mmT_

                emit(A_TILES, (0, 1))
                emit(B_TILES, (0,))
                s4A = reduce_chunk(0, "A")
                emit(B_TILES[:8], (1,))
                s2A = fold_chunk(0, "A", s4A)
                emit(B_TILES[8:16], (1,))
                mmTA = transpose_chunk("A", s2A)
                emit(B_TILES[16:], (1,))

                # names for per-half state pieces of this iteration
                rec_new = spool.tile([P, F], F32, tag="recfull")
                r_new = spool.tile([P, F], F32, tag="r")
                recS_new = spool.tile([P, F], F32, tag="recS")
                q = spool.tile([P, F], F32, tag="u")
                v = spool.tile([P, F], F32, tag="x")
                newy = {"A": yp.tile([P, HA], F8, tag="yA", name=f"yA_{it}"),
                        "B": yp.tile([P, HB], F8, tag="yB", name=f"yB_{it}")}

                def ew_half(hf, mmT_half):
                    """Element-wise update chain, column-split across DVE
                    (cols 0:EW_DVE) and Pool (cols EW_DVE:) so the two
                    engines run the serial chain concurrently."""
                    sl = slice(0, HA) if hf == "A" else slice(HA, F)
                    HF = HW[hf]
                    h_ = wk.tile([P, HF], F32, tag=f"w1{hf}", bufs=1)
                    dr_ = wk.tile([P, HF], F32, tag=f"w2{hf}", bufs=1)
                    m1_ = wk.tile([P, HF], F32, tag=f"w3{hf}", bufs=1)
                    tt_ = wk.tile([P, HF], F32, tag=f"w4{hf}", bufs=1)
                    s2_ = wk.tile([P, HF], F32, tag=f"w5{hf}", bufs=1)
                    tmp = wk.tile([P, HF], F32, tag=f"w0{hf}", bufs=1)
                    ynew = None if last else newy[hf]

                    # h reads mmT from PSUM, which Pool cannot access --
                    # compute it (and the general-path rec chain) on DVE
                    # full-width, then split the rest across DVE/Pool.
                    if fast_rec:
                        V.tensor_add(h_[:], mmT_half[:], rff[:, sl])
                    elif ds_v is not None:
                        V.tensor_add(rec_new[:, sl], mmT_half[:], recS[:, sl])
                        V.tensor_add(h_[:], rec_new[:, sl], ff_sb[:, sl])
                    else:
                        V.tensor_mul(tmp[:], mmT_half[:], ds_sb[:, sl])
                        V.tensor_add(rec_new[:, sl], tmp[:], recS[:, sl])
                        V.tensor_add(h_[:], rec_new[:, sl], ff_sb[:, sl])

                    def chain(E, i0, i1):
                        """TT/TS-only chain (Pool-compatible). C_t/D_t hold
                        YS*C and -YS*D, and v holds vY = YS*x, so y is the
                        plain product tt*vY."""
                        il = slice(i0, i1)           # intermediate cols
                        gl = slice(sl.start + i0, sl.start + i1)  # state cols
                        E.tensor_scalar(dr_[:, il], h_[:, il], 0.0, dt_v,
                                        MAX, MULT)
                        E.tensor_add(r_new[:, gl], dr_[:, il], rE[:, gl])
                        if last:
                            return
                        E.tensor_mul(m1_[:, il], B_t[:, gl], r_new[:, gl])
                        E.tensor_add(q[:, gl], m1_[:, il], A_t[:, gl])
                        E.tensor_mul(tt_[:, il], r_new[:, gl], q[:, gl])
                        E.tensor_mul(s2_[:, il], D_t[:, gl], tt_[:, il])
                        E.tensor_add(v[:, gl], s2_[:, il], C_t[:, gl])
                        E.tensor_mul(ynew[:, il], tt_[:, il], v[:, gl])

                    def chain_dve(i0, i1):
                        """general-path chain on DVE only (STT forms)."""
                        il = slice(i0, i1)
                        gl = slice(sl.start + i0, sl.start + i1)
                        if dt_v is not None:
                            V.tensor_scalar(dr_[:, il], h_[:, il], 0.0, dt_v,
                                            MAX, MULT)
                        else:
                            V.scalar_tensor_tensor(dr_[:, il], h_[:, il], 0.0,
                                                   dt_sb[:, gl], MAX, MULT)
                        if e_v is not None:
                            V.scalar_tensor_tensor(r_new[:, gl], r[:, gl],
                                                   e_v, dr_[:, il], MULT, ADD)
                        else:
                            V.tensor_add(r_new[:, gl], dr_[:, il], rE[:, gl])
                        if last:
                            return
                        V.tensor_mul(m1_[:, il], B_t[:, gl], r_new[:, gl])
                        V.tensor_add(q[:, gl], m1_[:, il], A_t[:, gl])
                        V.tensor_mul(tt_[:, il], r_new[:, gl], q[:, gl])
                        V.tensor_mul(s2_[:, il], D_t[:, gl], tt_[:, il])
                        V.scalar_tensor_tensor(v[:, gl], s2_[:, il], -1.0,
                                               C_t[:, gl], MULT, ADD)
                        V.scalar_tensor_tensor(ynew[:, il], tt_[:, il], YS,
                                               v[:, gl], MULT, MULT)

                    if pool_split:
                        chain(V, 0, min(EW_DVE, HF))
                        if EW_DVE < HF:
                            chain(nc.gpsimd, EW_DVE, HF)
                    else:
                        chain_dve(0, HF)
                    # off the y critical path: recS' update, full-width DVE
                    if not last:
                        if fast_rec:
                            # recS' = es*(h - ff) = es*h + (-es*ff)
                            V.scalar_tensor_tensor(recS_new[:, sl], h_[:],
                                                   es_v, negesff[:, sl],
                                                   MULT, ADD)
                        elif es_v is not None:
                            V.tensor_scalar(recS_new[:, sl], rec_new[:, sl],
                                            es_v, None, MULT)
                        else:
                            V.tensor_mul(recS_new[:, sl], rec_new[:, sl],
                                         es_sb[:, sl])
                    return ynew

                yA_next = ew_half("A", mmTA)
                if not last:
                    sA = ag_send("A", yA_next)

                s4B = reduce_chunk(1, "B")
                s2B = fold_chunk(1, "B", s4B)
                mmTB = transpose_chunk("B", s2B)
                yB_next = ew_half("B", mmTB)
                if not last:
                    sB = ag_send("B", yB_next)
                    yfullA = ag_recv("A", *sA)
                    yfullB = ag_recv("B", *sB)
                    un, xn, recS = q, v, recS_new
                r = r_new

            # ---- epilogue ----
            for qi in range(4):
                nc.sync.dma_start(
                    r_out[32 * qi:32 * (qi + 1), :],
                    r[32 * qi:32 * (qi + 1), :],
                )

    nc.compile()
    return nc


# ---------------------------------------------------------------------------
# host-side data marshalling
# ---------------------------------------------------------------------------

def _shard_state(v, c):
    """[B, N] float array -> core c state tile [128, 256] (f32)."""
    vs = np.asarray(v, np.float32)[:, c * NS:(c + 1) * NS]      # [32, 1024]
    return np.ascontiguousarray(
        vs.reshape(B, J, P).transpose(2, 1, 0).reshape(P, F)
    )


def _shard_vec(v, c):
    """[N] float vector -> replicated core c tile [128, 256] (f32)."""
    vs = np.asarray(v, np.float32)[c * NS:(c + 1) * NS].reshape(J, P)  # [j, p]
    t = vs.T[:, :, None]                                        # [p, j, 1]
    return np.ascontiguousarray(np.broadcast_to(t, (P, J, B)).reshape(P, F))


def _shard_w(Wab, c):
    """Wab [N, N] -> core c weight tiles [64, 128, 1024] fp16.

    w[t, p, n] = Wab[c*1024 + n, t*128 + p]
    """
    wt = np.asarray(Wab, np.float32)[c * NS:(c + 1) * NS, :].T  # [8192, 1024]
    return np.ascontiguousarray(wt.astype(np.float16).reshape(T, P, NS))


def _fold_mat(ds_v):
    """[128, 32] fp16: fold[32s+m, m] = ds/YS (strip-sum + ds scale +
    removal of the fp8 y scale YS)."""
    f = np.zeros((P, B), np.float16)
    scale = np.float16((1.0 if ds_v is None else ds_v) / YS)
    for s in range(S):
        f[s * B:(s + 1) * B, :][np.diag_indices(B)] = scale
    return f


def _unshard_out(tiles):
    """list of 8 [128, 256] tiles -> [32, 8192] f32."""
    out = np.empty((B, N), np.float32)
    for c, tl in enumerate(tiles):
        out[:, c * NS:(c + 1) * NS] = (
            np.asarray(tl, np.float32).reshape(P, J, B).transpose(2, 1, 0)
            .reshape(B, NS)
        )
    return out


def make_in_maps(rates, rec_input, ff_input, Wab, u_stp, x_stp,
                 exp_dt_tau, dt_tau, exp_dt_tau_syn, dt_tau_syn):
    recs_full = (np.asarray(exp_dt_tau_syn, np.float32)[None, :]
                 * np.asarray(rec_input, np.float32))
    ds_v = _uniform_val(dt_tau_syn)
    fold = _fold_mat(ds_v)
    identh = np.eye(B, dtype=np.float16)
    in_maps = []
    for c in range(NCORES):
        in_maps.append({
            "w": _shard_w(Wab, c),
            "r0": _shard_state(rates, c),
            "recs0": _shard_state(recs_full, c),
            "u0": _shard_state(u_stp, c),
            "x0": _shard_state(x_stp, c),
            "ff": _shard_state(ff_input, c),
            "es": _shard_vec(exp_dt_tau_syn, c),
            "ds": _shard_vec(dt_tau_syn, c),
            "e": _shard_vec(exp_dt_tau, c),
            "dt": _shard_vec(dt_tau, c),
            "fold": fold,
            "identh": identh,
        })
    return in_maps


_PROGRAM_CACHE = {}


def _uniform_val(v):
    v = np.asarray(v, np.float32)
    return float(v.flat[0]) if np.all(v == v.flat[0]) else None


def _get_program(n_steps, uni):
    key = (n_steps, uni)
    if key not in _PROGRAM_CACHE:
        _PROGRAM_CACHE[key] = build_program(n_steps, uni=uni)
    return _PROGRAM_CACHE[key]


def run(trace=False, tmpdir=None, **inputs):
    n_steps = int(inputs.pop("n_steps"))
    uni = (_uniform_val(inputs["exp_dt_tau_syn"]),
           _uniform_val(inputs["dt_tau_syn"]),
           _uniform_val(inputs["exp_dt_tau"]),
           _uniform_val(inputs["dt_tau"]))
    nc = _get_program(n_steps, uni)
    in_maps = make_in_maps(**inputs)
    res = bass_utils.run_bass_kernel_spmd(
        nc, in_maps, core_ids=list(range(NCORES)), trace=trace, tmpdir=tmpdir
    )
    out = _unshard_out([m["r_out"] for m in res.results])
    return out, res


def kernel(**inputs):
    out, _ = run(**inputs)
    return out

